# revision 45
# baseline (speedup 1.0000x reference)
"""Multi-head attention with LoRA adapters on 8 Trainium2 NeuronCores.

Problem: x[4,2048,768] -> LoRA-linear QKV -> 12-head attention -> LoRA-linear out proj.

Math notes:
  - LoRA folded into base weights on host: x@(W + B@A).T + b (exact).
  - bv folded into output bias via softmax(row)@1 == 1.
  - Softmax without max-subtraction; row sum rides as a ones column in v
    (M=65 PV matmuls); division applied to the tiny PV output.

Sharding: core = 2*b + g for batch b, head-group g (6 heads each); host sums
the two bf16 row-sharded output-projection partials per batch.

v2 schedule (ACT exp is the pacer; scores row-tiled for PE concurrency):
  - Unit = (cb, jc): head-PAIR cb (heads 2cb/2cb+1 at partitions 0-63/64-127)
    x 512-col j-chunk jc. 12 units x 16 steps. Per step s: scores for BOTH
    heads into psB[128,1024] halves — two K=64 matmuls at tile_position (0,0)
    and (64,0), which the PE runs CONCURRENTLY (row tiling; auto-derived from
    base partitions) — then ONE exp [128,1024] on ACT (1038ns, the step
    clock), then 2 PV matmuls (K=128) into per-head psC[65,512] banks.
  - PV emitted 2 steps behind its exp; drains at unit end: per head,
    psC -> SBUF stage, DVE recip row 64, Pool broadcast, DVE mul into outT.
  - PSUM (8 banks): psB 2x[128,1024] (4) + psC 2x[65,512] (2) + psp (2).
  - DMA: all triggers on the sync queue (HWDGE is serial, ~0.62us/DMA;
    triggers park the issuing sequencer, so the ACT queue must stay clean);
    weights partition-major (cb0 k+q combined in one wkq0 tensor, loaded
    first), xT j-striped in waves; ident warmup matmuls (memset tile) hold
    the DVFS ramp until wave 0 lands. The exp stream starts ~9.5us in (the
    lead + unit 0 run at the per-core HBM roofline for the 5.4MB inputs). Everything else (k/q jt1-3, v, later
    cbs, oproj partials, epilogue) is demand-marked filler inside the exp
    stream, front-loaded in unit 0 to meet the kT/v step deadlines.
  - Drains: bf16 stage (frees psC bank; normalize mul hits the DVE 4x bf16
    mode; denominators rounded to bf16, ~+0.9e-3 on the error, gate 2e-2).
  - Epilogue: oproj partials staged bf16 (sOutA/sOutB); cb2 term + identity
    fold per mt; mt batches attach to cb2 units as their j-chunks drain, so
    only mt 12-15 gate the tail: their drain runs as 256-col chunks per mt
    pair (recip/bcast/mul straight from PSUM, minimizing cross-engine sem
    hops) feeding the epi_fulls immediately; the last tile's copy+DMA is
    split in halves.

TimelineSim: 249.0us (vs 251.9 v1) — the cost model serializes row-tiled
matmuls, so the scores overlap (~41us of PE) is invisible to it; projecting
the sim exp cadence with the overlap credit gives ~232.5us on HW (lead 9.5
+ phase 211.2 + tail 11.8). Worst case (no overlap on HW) still beats v1.
"""

import sys

sys.path.insert(0, "/opt/trn_rl_repo")

import numpy as np

DIM, HEADS, R = 768, 12, 8
B, T = 4, 2048
HD = DIM // HEADS          # 64 head dim
NCORES = 8
HG = HEADS // 2            # 6 heads per core
CS = HG * HD               # 384 local channels per core
SCALE = HD ** -0.5

_PROGRAM_CACHE = {}


def _bf16(a):
    import ml_dtypes
    return np.ascontiguousarray(a).astype(ml_dtypes.bfloat16)


def _pack_w(wT):
    # [DIM, CS] (row d = k*128+p, col m = cb*128+mi) -> [p, cb, k, mi] flat
    KB, CB = DIM // 128, CS // 128
    return np.ascontiguousarray(
        wT.reshape(KB, 128, CB, 128).transpose(1, 2, 0, 3).reshape(128, -1))


class _Filler:
    """Paced stream of independent PE work interleaved into attention steps.
    Markers allow demand-driven forcing (pace_until) for items whose results
    an upcoming attention matmul depends on."""

    def __init__(self):
        self.items = []          # (cols, fn)
        self.total = 0
        self.pos = 0
        self.done = 0

    def add(self, cols, fn):
        self.items.append((cols, fn))
        self.total += cols

    def mark(self):
        return len(self.items) - 1

    def pace(self, frac):
        target = self.total * min(frac, 1.0)
        while self.pos < len(self.items) and self.done < target:
            cols, fn = self.items[self.pos]
            fn()
            self.done += cols
            self.pos += 1

    def pace_until(self, idx):
        while self.pos <= idx:
            cols, fn = self.items[self.pos]
            fn()
            self.done += cols
            self.pos += 1

    def flush(self):
        self.pace(2.0)


def _build_program():
    import concourse.bass as bass
    import concourse.mybir as mybir
    import concourse.tile as tile
    from concourse import bacc

    f32 = mybir.dt.float32
    bf16 = mybir.dt.bfloat16

    nc = bacc.Bacc("TRN2", target_bir_lowering=False, debug=False,
                   num_devices=NCORES)

    xT = nc.dram_tensor("xT", [DIM, T], bf16, kind="ExternalInput")
    # wq/wk/wv arrive partition-major [p, cb, k, m]: the cb0 slice and the
    # cb1-2 rest are each one contiguous-run-per-partition DMA (128 x 1.5KB
    # descriptors instead of 768 x 256B -> ~2x DMA rate on the lead-in)
    wq_t = nc.dram_tensor("wq_t", [128, CS * DIM // 128], bf16,
                          kind="ExternalInput")
    wk_t = nc.dram_tensor("wk_t", [128, CS * DIM // 128], bf16,
                          kind="ExternalInput")
    wv_t = nc.dram_tensor("wv_t", [128, CS * DIM // 128], bf16,
                          kind="ExternalInput")
    wkq0 = nc.dram_tensor("wkq0", [128, 2 * DIM], bf16, kind="ExternalInput")
    wo_t = nc.dram_tensor("wo_t", [CS, DIM], bf16, kind="ExternalInput")
    bq_s = nc.dram_tensor("bq_s", [CS], f32, kind="ExternalInput")
    bk_s = nc.dram_tensor("bk_s", [CS], f32, kind="ExternalInput")
    bo_s = nc.dram_tensor("bo_s", [DIM], f32, kind="ExternalInput")
    ident_d = nc.dram_tensor("ident", [128, 128], bf16, kind="ExternalInput")
    out_p = nc.dram_tensor("out_p", [T, DIM], bf16, kind="ExternalOutput")

    KB = DIM // 128      # 6 k-blocks of the input dim
    CB = CS // 128       # 3 channel blocks (head pairs)
    TB = T // 128        # 16 s tiles
    NJC = T // 512       # 4 j chunks per unit sweep
    VW = HD + 1          # 65: v plus ones column

    with tile.TileContext(nc) as tc:
        with (
            tc.tile_pool(name="weights", bufs=1) as wpool,
            tc.tile_pool(name="psB", bufs=2, space="PSUM") as psB_pool,
            tc.tile_pool(name="psC", bufs=2, space="PSUM") as psC_pool,
            tc.tile_pool(name="psp", bufs=2, space="PSUM") as psp_pool,
            tc.tile_pool(name="epool", bufs=5) as e_pool,
            tc.tile_pool(name="stage", bufs=3) as st_pool,
            tc.tile_pool(name="npool", bufs=4) as n_pool,
            tc.tile_pool(name="opool", bufs=8) as o_pool,
        ):
            # ---- inputs: channel-sliced weights + j-striped xT so the cb0
            # jt0 prologue completes in ~4us ----
            w_kq0 = wpool.tile([128, 2, KB, 128], bf16)
            w_kt = wpool.tile([128, KB, CS], bf16)
            w_qt = wpool.tile([128, KB, CS], bf16)
            w_vt = wpool.tile([128, KB, CS], bf16)
            xT_sb = wpool.tile([128, KB, T], bf16)
            wk_view = wk_t.ap().rearrange("p (c k m) -> p c k m", c=CB, k=KB)
            wq_view = wq_t.ap().rearrange("p (c k m) -> p c k m", c=CB, k=KB)
            wv_view = wv_t.ap().rearrange("p (c k m) -> p c k m", c=CB, k=KB)
            xT_view = xT.ap().rearrange("(k p) t -> p k t", p=128)
            bq_sb = wpool.tile([128, CB], f32)
            bk_sb = wpool.tile([128, CB], f32)
            w_ot = wpool.tile([128, CB, DIM], bf16)
            bo_row = wpool.tile([1, DIM], f32)
            ident = wpool.tile([128, 128], bf16)
            # ALL DMAs ride the sync queue in need-order; the ACT queue stays
            # clean (a DMA trigger parks its sequencer on the serial HWDGE,
            # which would stall the exp stream behind weight loads).
            nc.sync.dma_start(
                out=w_kq0, in_=wkq0.ap().rearrange("p (w k m) -> p w k m",
                                                   w=2, k=KB))
            nc.sync.dma_start(out=xT_sb[:, 0:3, 0:512], in_=xT_view[:, 0:3, 0:512])
            nc.sync.dma_start(out=xT_sb[:, 3:6, 0:512], in_=xT_view[:, 3:6, 0:512])
            nc.sync.dma_start(
                out=bk_sb, in_=bk_s.ap().rearrange("(k p) -> p k", p=128))
            nc.sync.dma_start(
                out=bq_sb, in_=bq_s.ap().rearrange("(k p) -> p k", p=128))
            nc.sync.dma_start(out=xT_sb[:, :, 512:1024], in_=xT_view[:, :, 512:1024])
            nc.sync.dma_start(out=w_vt[:, :, 0:128], in_=wv_view[:, 0])
            nc.sync.dma_start(out=xT_sb[:, :, 1024:1536],
                              in_=xT_view[:, :, 1024:1536])
            nc.sync.dma_start(out=xT_sb[:, :, 1536:2048],
                              in_=xT_view[:, :, 1536:2048])
            for cb in (1, 2):
                nc.sync.dma_start(out=w_kt[:, :, cb * 128:cb * 128 + 128],
                                  in_=wk_view[:, cb])
                nc.sync.dma_start(out=w_qt[:, :, cb * 128:cb * 128 + 128],
                                  in_=wq_view[:, cb])
                nc.sync.dma_start(out=w_vt[:, :, cb * 128:cb * 128 + 128],
                                  in_=wv_view[:, cb])
            nc.sync.dma_start(
                out=w_ot, in_=wo_t.ap().rearrange("(k p) m -> p k m", p=128))
            nc.sync.dma_start(out=ident, in_=ident_d.ap())
            nc.sync.dma_start(out=bo_row,
                              in_=bo_s.ap().rearrange("(o d) -> o d", o=1))
            bo_sb = wpool.tile([128, DIM], f32)
            nc.gpsimd.partition_broadcast(bo_sb, bo_row)
            # PE warmup: ident@ident matmuls hold the DVFS busy-streak from
            # ~1.5us until the wave-0 xT stripes land, so the prologue and
            # attention run at full clock from the first real matmul.
            wsrc = wpool.tile([128, 128], bf16)
            nc.vector.memset(wsrc.bitcast(mybir.dt.uint16), 0x3F80)
            warm = psp_pool.tile([128, 512], f32, tag="psp", name="warm")
            for _ in range(40):
                nc.tensor.matmul(warm[:, 0:128], wsrc, wsrc,
                                 start=True, stop=True)

            # ---- persistent activations ----
            qT_sb = wpool.tile([128, CB, T], bf16)
            kT_sb = wpool.tile([128, CB, T], bf16)
            v_sb = wpool.tile([128, TB, HG * VW], bf16)
            outT_sb = wpool.tile([128, CB, T], bf16)
            sOutA = wpool.tile([128, TB, DIM], bf16)
            sOutB = wpool.tile([128, TB, DIM], bf16)

            # ones columns of v_aug (one strided memset)
            ones_ap = bass.AP(
                tensor=v_sb.tensor, offset=v_sb.offset + HD,
                ap=[v_sb.ap[0], [HG * VW, TB], [VW, HG]],
            )
            nc.vector.memset(ones_ap.bitcast(mybir.dt.uint16), 0x3F80)

            # ---- prologue: k(cb0) jt0 + q(cb0) jt0 in one psB tile,
            # kk-OUTER so the PE tracks the xT wave-0 DMA stripes ----
            proAcc = psB_pool.tile([128, 1024], f32, tag="psB", name="proAcc")
            for kk in range(KB):
                nc.tensor.matmul(proAcc[:, 0:512], w_kq0[:, 0, kk, :],
                                 xT_sb[:, kk, 0:512],
                                 start=(kk == 0), stop=(kk == KB - 1))
                nc.tensor.matmul(proAcc[:, 512:1024], w_kq0[:, 1, kk, :],
                                 xT_sb[:, kk, 0:512],
                                 start=(kk == 0), stop=(kk == KB - 1))
            # NOTE: these must stay on DVE — gpsimd/Pool cannot read PSUM
            # (walrus birverifier rejects it; TimelineSim does not catch it)
            nc.vector.tensor_scalar_add(kT_sb[:, 0, 0:128], proAcc[:, 0:128],
                                        bk_sb[:, 0:1])
            nc.vector.tensor_scalar_add(qT_sb[:, 0, 0:512], proAcc[:, 512:1024],
                                        bq_sb[:, 0:1])
            nc.vector.tensor_scalar_add(kT_sb[:, 0, 128:512], proAcc[:, 128:512],
                                        bk_sb[:, 0:1])
            # preload the exp table off the critical path
            scr = wpool.tile([1, CB], f32)
            nc.scalar.activation(scr, bq_sb[0:1, :],
                                 mybir.ActivationFunctionType.Exp)

            # ---- filler work generators ----
            kmark, qmark, vmark, omark = {}, {}, {}, {}

            def kq_quanta(filler, w, dst, bias, cb, jts, markd):
                for jt in jts:
                    state = {}
                    for kk in range(KB):
                        def fn(jt=jt, kk=kk, state=state, cb=cb):
                            if kk == 0:
                                state["t"] = psp_pool.tile(
                                    [128, 512], f32, tag="psp", name="pspq")
                            cols = slice(jt * 512, (jt + 1) * 512)
                            nc.tensor.matmul(
                                state["t"], w[:, kk, cb * 128:(cb + 1) * 128],
                                xT_sb[:, kk, cols],
                                start=(kk == 0), stop=(kk == KB - 1),
                            )
                            if kk == KB - 1:
                                nc.vector.tensor_scalar_add(
                                    dst[:, cb, cols], state["t"],
                                    bias[:, cb:cb + 1])
                        filler.add(512, fn)
                    markd[(cb, jt)] = (filler, filler.mark())

            def emit_v_scatter(psp, st0, nst, c0, nch):
                for i in range(nst):
                    src = psp[:, i * nch:(i + 1) * nch].rearrange(
                        "p (h c) -> p h c", c=HD)
                    dst = bass.AP(
                        tensor=v_sb.tensor,
                        offset=v_sb.offset + (st0 + i) * (HG * VW) + (c0 // HD) * VW,
                        ap=[v_sb.ap[0], [VW, nch // HD], [1, HD]],
                    )
                    nc.vector.tensor_copy(dst, src)

            def v_quanta(filler, cb):
                c0, nch, nst = cb * 128, 128, 4
                for st0 in range(0, TB, nst):
                    state = {}
                    nmm = nst * KB
                    for m in range(nmm):
                        def fn(m=m, st0=st0, state=state):
                            if m == 0:
                                state["t"] = psp_pool.tile(
                                    [128, nst * nch], f32, tag="psp", name="pspv")
                            i, kk = divmod(m, KB)
                            nc.tensor.matmul(
                                state["t"][:, i * nch:(i + 1) * nch],
                                xT_sb[:, kk, (st0 + i) * 128:(st0 + i + 1) * 128],
                                w_vt[:, kk, c0:c0 + nch],
                                start=(kk == 0), stop=(kk == KB - 1),
                            )
                            if m == nmm - 1:
                                emit_v_scatter(state["t"], st0, nst, c0, nch)
                        filler.add(nch, fn)
                    idx = filler.mark()
                    for st in range(st0, st0 + nst):
                        vmark[(cb, st)] = (filler, idx)

            def oproj_quanta(filler, cb, dst, addend, markd=None):
                # dst[mt] = outT[cb]^T @ wo[cb] + addend (bf16 staging)
                for mt in range(TB):
                    for half in range(2):
                        cols = slice(half * 384, half * 384 + 384)
                        def fn(mt=mt, cols=cols, cb=cb):
                            psp = psp_pool.tile([128, 384], f32, tag="psp",
                                                name="pspo")
                            nc.tensor.matmul(
                                psp, outT_sb[:, cb, mt * 128:(mt + 1) * 128],
                                w_ot[:, cb, cols], start=True, stop=True,
                            )
                            if addend is sOutA:
                                nc.vector.tensor_add(
                                    dst[:, mt, cols], psp, addend[:, mt, cols])
                            else:
                                nc.vector.tensor_add(dst[:, mt, cols], psp,
                                                     addend[:, cols])
                        filler.add(384, fn)
                        if markd is not None:
                            markd[(mt, half)] = (filler, filler.mark())

            def emit_pv(item):
                s, e, psC0, psC1, cb = item
                for hb, psC in ((0, psC0), (1, psC1)):
                    h = 2 * cb + hb
                    nc.tensor.matmul(
                        psC, v_sb[:, s, h * VW:(h + 1) * VW],
                        e[:, hb * 512:hb * 512 + 512],
                        start=(s == 0), stop=(s == TB - 1),
                    )

            def emit_drain(item, post_hb=None, direct=False):
                # per head: psC -> SBUF stage (frees the bank), DVE recip of
                # row 64, Pool partition_broadcast, DVE mul into outT.
                psC0, psC1, cb, jc = item
                jcols = slice(jc * 512, jc * 512 + 512)
                for hb, psC in ((0, psC0), (1, psC1)):
                    if direct:
                        # fp32 straight from PSUM (tail path)
                        nr0 = n_pool.tile([1, 512], f32, tag="nr0")
                        nrb = n_pool.tile([HD, 512], f32, tag="nrb")
                        nc.vector.reciprocal(nr0, psC[HD:VW, :])
                        nc.gpsimd.partition_broadcast(nrb, nr0)
                        nc.vector.tensor_mul(
                            outT_sb[64 * hb:64 * hb + 64, cb, jcols],
                            psC[0:HD, :], nrb)
                    else:
                        # bf16 stage: frees the PSUM bank AND makes the
                        # normalize mul eligible for the DVE 4x bf16 mode
                        stage = st_pool.tile([VW, 512], bf16, tag="st")
                        nc.vector.tensor_copy(stage, psC)
                        nr0 = n_pool.tile([1, 512], bf16, tag="nr0")
                        nrb = n_pool.tile([HD, 512], bf16, tag="nrb")
                        with nc.allow_low_precision(
                                reason="softmax denom at bf16; output is "
                                       "bf16 anyway, ~0.4% on the row scale"):
                            nc.vector.reciprocal(nr0, stage[HD:VW, :])
                            nc.gpsimd.partition_broadcast(nrb, nr0)
                            nc.vector.tensor_mul(
                                outT_sb[64 * hb:64 * hb + 64, cb, jcols],
                                stage[0:HD, :], nrb)
                    if post_hb is not None:
                        post_hb(hb)

            epi_osb = {}

            def emit_epi(mt, half):
                # cb2 projection term + staged partials -> bf16 partial out
                m = omark.get((mt, half))
                if m is not None:
                    m[0].pace_until(m[1])
                cols = slice(half * 384, half * 384 + 384)
                psp = psp_pool.tile([128, 384], f32, tag="psp", name="pspe")
                nc.tensor.matmul(
                    psp, outT_sb[:, 2, mt * 128:(mt + 1) * 128],
                    w_ot[:, 2, cols], start=True, stop=True,
                )
                if half == 0:
                    epi_osb[mt] = o_pool.tile([128, DIM], bf16, tag="osb",
                                              name="osb")
                osb = epi_osb[mt]
                nc.vector.tensor_add(osb[:, cols], psp, sOutB[:, mt, cols])
                if half == 1:
                    nc.sync.dma_start(
                        out=out_p.ap()[mt * 128:(mt + 1) * 128, :], in_=osb)
                    del epi_osb[mt]

            def emit_epi_full(mt, use_act, split=False):
                # cb2 term + identity fold of staged partials; the copy out of
                # PSUM alternates ACT/DVE so the two engines split the tail.
                # split=True (last tile only): per-half copy+DMA so the final
                # wire-out overlaps the remaining matmuls.
                psD = psB_pool.tile([128, DIM], f32, tag="psB", name="psD")
                osb = o_pool.tile([128, DIM], bf16, tag="osb", name="osbf")
                eng = nc.scalar.copy if use_act else nc.vector.tensor_copy
                for half, cols in enumerate((slice(0, 512), slice(512, DIM))):
                    nc.tensor.matmul(
                        psD[:, cols], outT_sb[:, 2, mt * 128:(mt + 1) * 128],
                        w_ot[:, 2, cols], start=True, stop=False,
                    )
                    nc.tensor.matmul(
                        psD[:, cols], ident, sOutB[:, mt, cols],
                        start=False, stop=True,
                    )
                    if split:
                        eng(osb[:, cols], psD[:, cols])
                        nc.sync.dma_start(
                            out=out_p.ap()[mt * 128:(mt + 1) * 128, cols],
                            in_=osb[:, cols])
                if not split:
                    eng(osb, psD)
                    nc.sync.dma_start(
                        out=out_p.ap()[mt * 128:(mt + 1) * 128, :], in_=osb)

            # ---- filler streams ----
            f0 = _Filler()
            # cb0 k/q weights come from the combined w_kq0 tile (loaded first)
            kq_quanta(f0, w_kq0[:, 0], kT_sb, bk_sb, 0, (1,), kmark)
            v_quanta(f0, 0)                     # v(cb0) g0-3, vmarks
            kq_quanta(f0, w_kq0[:, 0], kT_sb, bk_sb, 0, (2, 3), kmark)
            kq_quanta(f0, w_kq0[:, 1], qT_sb, bq_sb, 0, (1, 2, 3), qmark)
            kq_quanta(f0, w_kt, kT_sb, bk_sb, 1, (0, 1, 2, 3), kmark)
            kq_quanta(f0, w_qt, qT_sb, bq_sb, 1, (0,), qmark)
            v_quanta(f0, 1)
            f1 = _Filler()
            kq_quanta(f1, w_qt, qT_sb, bq_sb, 1, (1, 2, 3), qmark)
            kq_quanta(f1, w_kt, kT_sb, bk_sb, 2, (0, 1, 2, 3), kmark)
            kq_quanta(f1, w_qt, qT_sb, bq_sb, 2, (0,), qmark)
            v_quanta(f1, 2)
            kq_quanta(f1, w_qt, qT_sb, bq_sb, 2, (1, 2, 3), qmark)
            f2a = _Filler()
            oproj_quanta(f2a, 0, sOutA, bo_sb)
            f2b = _Filler()
            oproj_quanta(f2b, 1, sOutB, sOutA, omark)

            # ---- attention: 12 units of (head-pair cb, 512-col jc) ----
            pend = []       # (s, e, psC0, psC1, cb) awaiting PV
            dpend = []      # (psC0, psC1, cb, jc) awaiting drain
            epi_done = [0]  # halves emitted for mt 0-11 (24 total)

            def pop_pv():
                it = pend.pop(0)
                m = vmark.get((it[4], it[0]))
                if m is not None:
                    m[0].pace_until(m[1])
                emit_pv(it)
                if it[0] == TB - 1:
                    emit_drain(dpend.pop(0))

            gs = 0
            for cb in range(CB):
                for jc in range(NJC):
                    m = qmark.get((cb, jc))
                    if m is not None:
                        m[0].pace_until(m[1])
                    psC0 = psC_pool.tile([VW, 512], f32, tag="psC", name="psC0")
                    psC1 = psC_pool.tile([VW, 512], f32, tag="psC", name="psC1")
                    for s in range(TB):
                        if jc == 0 and s % 4 == 3 and s < 12:
                            # force k jt s//4+1: jt1 at s3, jt2 s7, jt3 s11
                            m = kmark.get((cb, s // 4 + 1))
                            if m is not None:
                                m[0].pace_until(m[1])
                        if s == 13:
                            # pre-force the NEXT unit's q chunk mid-unit so the
                            # burst doesn't collide with the boundary drains
                            nxt = (cb, jc + 1) if jc + 1 < NJC else (cb + 1, 0)
                            m = qmark.get(nxt)
                            if m is not None:
                                m[0].pace_until(m[1])
                        psB = psB_pool.tile([128, 1024], f32, tag="psB")
                        for hb in range(2):
                            prow = slice(64 * hb, 64 * hb + 64)
                            nc.tensor.matmul(
                                psB[:, hb * 512:hb * 512 + 512],
                                kT_sb[prow, cb, s * 128:(s + 1) * 128],
                                qT_sb[prow, cb, jc * 512:(jc + 1) * 512],
                                start=True, stop=True,
                            )
                        e = e_pool.tile([128, 1024], bf16, tag="e")
                        nc.scalar.activation(
                            e, psB, mybir.ActivationFunctionType.Exp,
                            scale=SCALE,
                        )
                        pend.append((s, e, psC0, psC1, cb))
                        thr = 1 if (cb == 2 and jc == 3) else 2
                        if len(pend) > thr:
                            pop_pv()
                            if len(pend) > thr:
                                pop_pv()
                        # background pacing (two half-steps for smoothness)
                        for ph in (0.5, 1.0):
                            if gs < 16:
                                f0.pace((gs + ph) / 36.0)
                            if gs < 80:
                                f0.pace((gs + ph) / 72.0)
                            if 80 <= gs < 136:
                                f1.pace((gs - 80 + ph) / 55.0)
                            if 68 <= gs < 133:
                                f2a.pace((gs - 68 + ph) / 64.0)
                            if gs >= 134:
                                f2b.pace((gs - 134 + ph) / 32.0)
                        # epilogue batches ride cb2 units jc>=1; the last 4
                        # halves (mt10-11) are held back to fill the PE hole
                        # under the final drain chain
                        if cb == 2 and jc >= 1 and s >= 4:
                            want = min(8, (s - 3) * 8 // 12)
                            cap = 22 if jc == 3 else (jc - 1) * 8 + 8
                            while epi_done[0] < min(cap, (jc - 1) * 8 + want):
                                emit_epi(epi_done[0] // 2, epi_done[0] % 2)
                                epi_done[0] += 1
                        gs += 1
                    dpend.append((psC0, psC1, cb, jc))
            f0.flush()
            f1.flush()
            f2a.flush()

            while pend:
                it = pend.pop(0)
                emit_pv(it)
                if it[0] == TB - 1:
                    # finale: the last drain runs as 256-col chunks (one per
                    # mt PAIR: recip/bcast/mul straight from PSUM, fewer
                    # cross-engine sem hops than per-mt) and each pair's
                    # epi_fulls start as soon as its columns are normalized
                    f2b.flush()
                    # mt11 held back from the stream: its identity-fold mms
                    # (no DVE coupling) fill the PE under the first drain chunk
                    emit_epi_full(11, use_act=True)
                    psC0f, psC1f, _cb, _jc = dpend.pop(0)
                    for mtp in (12, 14):
                        cc = slice((mtp - 12) * 128, (mtp - 12) * 128 + 256)
                        for hb, psC in ((0, psC0f), (1, psC1f)):
                            nr0 = n_pool.tile([1, 256], f32, tag="nr0")
                            nrb = n_pool.tile([HD, 256], f32, tag="nrb")
                            nc.vector.reciprocal(nr0, psC[HD:VW, cc])
                            nc.gpsimd.partition_broadcast(nrb, nr0)
                            nc.vector.tensor_mul(
                                outT_sb[64 * hb:64 * hb + 64, 2,
                                        mtp * 128:(mtp + 2) * 128],
                                psC[0:HD, cc], nrb)
                        emit_epi_full(mtp, use_act=(mtp % 2 == 1))
                        emit_epi_full(mtp + 1, use_act=(mtp % 2 == 0),
                                      split=(mtp == 14))


    nc.compile()
    return nc


def _get_program():
    if "nc" not in _PROGRAM_CACHE:
        _PROGRAM_CACHE["nc"] = _build_program()
    return _PROGRAM_CACHE["nc"]


def _prep_in_maps(inputs):
    f = np.float32
    inputs = {k: np.asarray(v) for k, v in inputs.items()}

    def eff(w, a, bl):
        return (w.astype(np.float64) + bl.astype(np.float64) @ a.astype(np.float64)).astype(f)

    wq = eff(inputs["wq"], inputs["laq"], inputs["lbq"])
    wk = eff(inputs["wk"], inputs["lak"], inputs["lbk"])
    wv = eff(inputs["wv"], inputs["lav"], inputs["lbv"])
    wo = eff(inputs["wo"], inputs["lao"], inputs["lbo"])
    x = np.asarray(inputs["x"], dtype=f)
    bq, bk, bv, bo = (np.asarray(inputs[k], dtype=f) for k in ("bq", "bk", "bv", "bo"))

    in_maps = []
    for core in range(NCORES):
        b, g = core // 2, core % 2
        cols = slice(g * CS, (g + 1) * CS)
        bo_core = wo[:, cols].astype(np.float64) @ bv[cols].astype(np.float64)
        if g == 0:
            bo_core = bo_core + bo
        in_maps.append({
            "xT": _bf16(x[b].T),
            "wq_t": _bf16(_pack_w(wq[cols, :].T)),
            "wkq0": _bf16(np.concatenate(
                [_pack_w(wk[cols, :].T)[:, 0:DIM],
                 _pack_w(wq[cols, :].T)[:, 0:DIM]], axis=1)),
            "wk_t": _bf16(_pack_w(wk[cols, :].T)),
            "wv_t": _bf16(_pack_w(wv[cols, :].T)),
            "wo_t": _bf16(wo[:, cols].T),
            "bq_s": np.ascontiguousarray(bq[cols]),
            "bk_s": np.ascontiguousarray(bk[cols]),
            "bo_s": bo_core.astype(f),
            "ident": _bf16(np.eye(128, dtype=f)),
        })
    return in_maps


def kernel(**inputs):
    from concourse.bass_utils import run_bass_kernel_spmd

    nc = _get_program()
    in_maps = _prep_in_maps(inputs)
    res = run_bass_kernel_spmd(nc, in_maps, core_ids=list(range(NCORES)))
    out = np.empty((B, T, DIM), dtype=np.float32)
    for b in range(B):
        out[b] = (res.results[2 * b]["out_p"].astype(np.float32)
                  + res.results[2 * b + 1]["out_p"].astype(np.float32))
    return out


# revision 46
# speedup vs baseline: 1.0007x; 1.0007x over previous
"""Multi-head attention with LoRA adapters on 8 Trainium2 NeuronCores.

Problem: x[4,2048,768] -> LoRA-linear QKV -> 12-head attention -> LoRA-linear out proj.

Math notes:
  - LoRA folded into base weights on host: x@(W + B@A).T + b (exact).
  - bv folded into output bias via softmax(row)@1 == 1.
  - Softmax without max-subtraction; row sum rides as a ones column in v
    (M=65 PV matmuls); division applied to the tiny PV output.

Sharding: core = 2*b + g for batch b, head-group g (6 heads each); host sums
the two bf16 row-sharded output-projection partials per batch.

v2 schedule (ACT exp is the pacer; scores row-tiled for PE concurrency):
  - Unit = (cb, jc): head-PAIR cb (heads 2cb/2cb+1 at partitions 0-63/64-127)
    x 512-col j-chunk jc. 12 units x 16 steps. Per step s: scores for BOTH
    heads into psB[128,1024] halves — two K=64 matmuls at tile_position (0,0)
    and (64,0), which the PE runs CONCURRENTLY (row tiling; auto-derived from
    base partitions) — then ONE exp [128,1024] on ACT (1038ns, the step
    clock), then 2 PV matmuls (K=128) into per-head psC[65,512] banks.
  - PV emitted 2 steps behind its exp; drains at unit end: per head,
    psC -> SBUF stage, DVE recip row 64, Pool broadcast, DVE mul into outT.
  - PSUM (8 banks): psB 2x[128,1024] (4) + psC 2x[65,512] (2) + psp (2).
  - DMA: all triggers on the sync queue (HWDGE is serial, ~0.62us/DMA;
    triggers park the issuing sequencer, so the ACT queue must stay clean);
    weights partition-major (cb0 k+q combined in one wkq0 tensor, loaded
    first), xT j-striped in waves; ident warmup matmuls (memset tile) hold
    the DVFS ramp until wave 0 lands. The exp stream starts ~9.5us in (the
    lead + unit 0 run at the per-core HBM roofline for the 5.4MB inputs). Everything else (k/q jt1-3, v, later
    cbs, oproj partials, epilogue) is demand-marked filler inside the exp
    stream, front-loaded in unit 0 to meet the kT/v step deadlines.
  - Drains: bf16 stage (frees psC bank; normalize mul hits the DVE 4x bf16
    mode; denominators rounded to bf16, ~+0.9e-3 on the error, gate 2e-2).
  - Epilogue: oproj partials staged bf16 (sOutA/sOutB); cb2 term + identity
    fold per mt; mt batches attach to cb2 units as their j-chunks drain, so
    only mt 12-15 gate the tail: their drain runs as 256-col chunks per mt
    pair (recip/bcast/mul straight from PSUM, minimizing cross-engine sem
    hops) feeding the epi_fulls immediately; the last tile's copy+DMA is
    split in halves.

TimelineSim: 249.0us (vs 251.9 v1) — the cost model serializes row-tiled
matmuls, so the scores overlap (~41us of PE) is invisible to it; projecting
the sim exp cadence with the overlap credit gives ~232.5us on HW (lead 9.5
+ phase 211.2 + tail 11.8). Worst case (no overlap on HW) still beats v1.
"""

import sys

sys.path.insert(0, "/opt/trn_rl_repo")

import numpy as np

DIM, HEADS, R = 768, 12, 8
B, T = 4, 2048
HD = DIM // HEADS          # 64 head dim
NCORES = 8
HG = HEADS // 2            # 6 heads per core
CS = HG * HD               # 384 local channels per core
SCALE = HD ** -0.5

_PROGRAM_CACHE = {}


def _bf16(a):
    import ml_dtypes
    return np.ascontiguousarray(a).astype(ml_dtypes.bfloat16)


def _pack_w(wT):
    # [DIM, CS] (row d = k*128+p, col m = cb*128+mi) -> [p, cb, k, mi] flat
    KB, CB = DIM // 128, CS // 128
    return np.ascontiguousarray(
        wT.reshape(KB, 128, CB, 128).transpose(1, 2, 0, 3).reshape(128, -1))


class _Filler:
    """Paced stream of independent PE work interleaved into attention steps.
    Markers allow demand-driven forcing (pace_until) for items whose results
    an upcoming attention matmul depends on."""

    def __init__(self):
        self.items = []          # (cols, fn)
        self.total = 0
        self.pos = 0
        self.done = 0

    def add(self, cols, fn):
        self.items.append((cols, fn))
        self.total += cols

    def mark(self):
        return len(self.items) - 1

    def pace(self, frac):
        target = self.total * min(frac, 1.0)
        while self.pos < len(self.items) and self.done < target:
            cols, fn = self.items[self.pos]
            fn()
            self.done += cols
            self.pos += 1

    def pace_until(self, idx):
        while self.pos <= idx:
            cols, fn = self.items[self.pos]
            fn()
            self.done += cols
            self.pos += 1

    def flush(self):
        self.pace(2.0)


def _build_program():
    import concourse.bass as bass
    import concourse.mybir as mybir
    import concourse.tile as tile
    from concourse import bacc

    f32 = mybir.dt.float32
    bf16 = mybir.dt.bfloat16

    nc = bacc.Bacc("TRN2", target_bir_lowering=False, debug=False,
                   num_devices=NCORES)

    xT = nc.dram_tensor("xT", [DIM, T], bf16, kind="ExternalInput")
    # wq/wk/wv arrive partition-major [p, cb, k, m]: the cb0 slice and the
    # cb1-2 rest are each one contiguous-run-per-partition DMA (128 x 1.5KB
    # descriptors instead of 768 x 256B -> ~2x DMA rate on the lead-in)
    wq_t = nc.dram_tensor("wq_t", [128, CS * DIM // 128], bf16,
                          kind="ExternalInput")
    wk_t = nc.dram_tensor("wk_t", [128, CS * DIM // 128], bf16,
                          kind="ExternalInput")
    wv_t = nc.dram_tensor("wv_t", [128, CS * DIM // 128], bf16,
                          kind="ExternalInput")
    wkq0 = nc.dram_tensor("wkq0", [128, 2 * DIM], bf16, kind="ExternalInput")
    wo_t = nc.dram_tensor("wo_t", [CS, DIM], bf16, kind="ExternalInput")
    bq_s = nc.dram_tensor("bq_s", [CS], f32, kind="ExternalInput")
    bk_s = nc.dram_tensor("bk_s", [CS], f32, kind="ExternalInput")
    bo_s = nc.dram_tensor("bo_s", [DIM], f32, kind="ExternalInput")
    ident_d = nc.dram_tensor("ident", [128, 128], bf16, kind="ExternalInput")
    out_p = nc.dram_tensor("out_p", [T, DIM], bf16, kind="ExternalOutput")

    KB = DIM // 128      # 6 k-blocks of the input dim
    CB = CS // 128       # 3 channel blocks (head pairs)
    TB = T // 128        # 16 s tiles
    NJC = T // 512       # 4 j chunks per unit sweep
    VW = HD + 1          # 65: v plus ones column

    with tile.TileContext(nc) as tc:
        with (
            tc.tile_pool(name="weights", bufs=1) as wpool,
            tc.tile_pool(name="psB", bufs=2, space="PSUM") as psB_pool,
            tc.tile_pool(name="psC", bufs=2, space="PSUM") as psC_pool,
            tc.tile_pool(name="psp", bufs=2, space="PSUM") as psp_pool,
            tc.tile_pool(name="epool", bufs=5) as e_pool,
            tc.tile_pool(name="stage", bufs=3) as st_pool,
            tc.tile_pool(name="npool", bufs=4) as n_pool,
            tc.tile_pool(name="opool", bufs=8) as o_pool,
        ):
            # ---- inputs: channel-sliced weights + j-striped xT so the cb0
            # jt0 prologue completes in ~4us ----
            w_kq0 = wpool.tile([128, 2, KB, 128], bf16)
            w_kt = wpool.tile([128, KB, CS], bf16)
            w_qt = wpool.tile([128, KB, CS], bf16)
            w_vt = wpool.tile([128, KB, CS], bf16)
            xT_sb = wpool.tile([128, KB, T], bf16)
            wk_view = wk_t.ap().rearrange("p (c k m) -> p c k m", c=CB, k=KB)
            wq_view = wq_t.ap().rearrange("p (c k m) -> p c k m", c=CB, k=KB)
            wv_view = wv_t.ap().rearrange("p (c k m) -> p c k m", c=CB, k=KB)
            xT_view = xT.ap().rearrange("(k p) t -> p k t", p=128)
            bq_sb = wpool.tile([128, CB], f32)
            bk_sb = wpool.tile([128, CB], f32)
            w_ot = wpool.tile([128, CB, DIM], bf16)
            bo_row = wpool.tile([1, DIM], f32)
            ident = wpool.tile([128, 128], bf16)
            # ALL DMAs ride the sync queue in need-order; the ACT queue stays
            # clean (a DMA trigger parks its sequencer on the serial HWDGE,
            # which would stall the exp stream behind weight loads).
            nc.sync.dma_start(
                out=w_kq0, in_=wkq0.ap().rearrange("p (w k m) -> p w k m",
                                                   w=2, k=KB))
            nc.sync.dma_start(out=xT_sb[:, 0:3, 0:512], in_=xT_view[:, 0:3, 0:512])
            nc.sync.dma_start(out=xT_sb[:, 3:6, 0:512], in_=xT_view[:, 3:6, 0:512])
            nc.sync.dma_start(
                out=bk_sb, in_=bk_s.ap().rearrange("(k p) -> p k", p=128))
            nc.sync.dma_start(
                out=bq_sb, in_=bq_s.ap().rearrange("(k p) -> p k", p=128))
            nc.sync.dma_start(out=xT_sb[:, :, 512:1024], in_=xT_view[:, :, 512:1024])
            nc.sync.dma_start(out=w_vt[:, :, 0:128], in_=wv_view[:, 0])
            nc.sync.dma_start(out=xT_sb[:, :, 1024:1536],
                              in_=xT_view[:, :, 1024:1536])
            nc.sync.dma_start(out=xT_sb[:, :, 1536:2048],
                              in_=xT_view[:, :, 1536:2048])
            for cb in (1, 2):
                nc.sync.dma_start(out=w_kt[:, :, cb * 128:cb * 128 + 128],
                                  in_=wk_view[:, cb])
                nc.sync.dma_start(out=w_qt[:, :, cb * 128:cb * 128 + 128],
                                  in_=wq_view[:, cb])
                nc.sync.dma_start(out=w_vt[:, :, cb * 128:cb * 128 + 128],
                                  in_=wv_view[:, cb])
            nc.sync.dma_start(
                out=w_ot, in_=wo_t.ap().rearrange("(k p) m -> p k m", p=128))
            nc.sync.dma_start(out=ident, in_=ident_d.ap())
            nc.sync.dma_start(out=bo_row,
                              in_=bo_s.ap().rearrange("(o d) -> o d", o=1))
            bo_sb = wpool.tile([128, DIM], f32)
            nc.gpsimd.partition_broadcast(bo_sb, bo_row)
            # PE warmup: ident@ident matmuls hold the DVFS busy-streak from
            # ~1.5us until the wave-0 xT stripes land, so the prologue and
            # attention run at full clock from the first real matmul.
            wsrc = wpool.tile([128, 128], bf16)
            nc.vector.memset(wsrc.bitcast(mybir.dt.uint16), 0x3F80)
            warm = psp_pool.tile([128, 512], f32, tag="psp", name="warm")
            for _ in range(40):
                nc.tensor.matmul(warm[:, 0:128], wsrc, wsrc,
                                 start=True, stop=True)

            # ---- persistent activations ----
            qT_sb = wpool.tile([128, CB, T], bf16)
            kT_sb = wpool.tile([128, CB, T], bf16)
            v_sb = wpool.tile([128, TB, HG * VW], bf16)
            outT_sb = wpool.tile([128, CB, T], bf16)
            sOutA = wpool.tile([128, TB, DIM], bf16)
            sOutB = wpool.tile([128, TB, DIM], bf16)

            # ones columns of v_aug (one strided memset)
            ones_ap = bass.AP(
                tensor=v_sb.tensor, offset=v_sb.offset + HD,
                ap=[v_sb.ap[0], [HG * VW, TB], [VW, HG]],
            )
            nc.vector.memset(ones_ap.bitcast(mybir.dt.uint16), 0x3F80)

            # ---- prologue: k(cb0) jt0 + q(cb0) jt0 in one psB tile,
            # kk-OUTER so the PE tracks the xT wave-0 DMA stripes ----
            proAcc = psB_pool.tile([128, 1024], f32, tag="psB", name="proAcc")
            for kk in range(KB):
                nc.tensor.matmul(proAcc[:, 0:512], w_kq0[:, 0, kk, :],
                                 xT_sb[:, kk, 0:512],
                                 start=(kk == 0), stop=(kk == KB - 1))
                nc.tensor.matmul(proAcc[:, 512:1024], w_kq0[:, 1, kk, :],
                                 xT_sb[:, kk, 0:512],
                                 start=(kk == 0), stop=(kk == KB - 1))
            # qT move on ACT (Identity+bias shares the exp table set, no
            # reload; ACT is idle pre-stream) in PARALLEL with kT on DVE.
            # gpsimd/Pool cannot read PSUM (walrus rejects; sim doesn't catch)
            nc.vector.tensor_scalar_add(kT_sb[:, 0, 0:128], proAcc[:, 0:128],
                                        bk_sb[:, 0:1])
            nc.scalar.add(qT_sb[:, 0, 0:512], proAcc[:, 512:1024],
                          bq_sb[:, 0:1])
            nc.vector.tensor_scalar_add(kT_sb[:, 0, 128:512], proAcc[:, 128:512],
                                        bk_sb[:, 0:1])
            # preload the exp table off the critical path
            scr = wpool.tile([1, CB], f32)
            nc.scalar.activation(scr, bq_sb[0:1, :],
                                 mybir.ActivationFunctionType.Exp)

            # ---- filler work generators ----
            kmark, qmark, vmark, omark = {}, {}, {}, {}

            def kq_quanta(filler, w, dst, bias, cb, jts, markd):
                for jt in jts:
                    state = {}
                    for kk in range(KB):
                        def fn(jt=jt, kk=kk, state=state, cb=cb):
                            if kk == 0:
                                state["t"] = psp_pool.tile(
                                    [128, 512], f32, tag="psp", name="pspq")
                            cols = slice(jt * 512, (jt + 1) * 512)
                            nc.tensor.matmul(
                                state["t"], w[:, kk, cb * 128:(cb + 1) * 128],
                                xT_sb[:, kk, cols],
                                start=(kk == 0), stop=(kk == KB - 1),
                            )
                            if kk == KB - 1:
                                nc.vector.tensor_scalar_add(
                                    dst[:, cb, cols], state["t"],
                                    bias[:, cb:cb + 1])
                        filler.add(512, fn)
                    markd[(cb, jt)] = (filler, filler.mark())

            def emit_v_scatter(psp, st0, nst, c0, nch):
                for i in range(nst):
                    src = psp[:, i * nch:(i + 1) * nch].rearrange(
                        "p (h c) -> p h c", c=HD)
                    dst = bass.AP(
                        tensor=v_sb.tensor,
                        offset=v_sb.offset + (st0 + i) * (HG * VW) + (c0 // HD) * VW,
                        ap=[v_sb.ap[0], [VW, nch // HD], [1, HD]],
                    )
                    nc.vector.tensor_copy(dst, src)

            def v_quanta(filler, cb):
                c0, nch, nst = cb * 128, 128, 4
                for st0 in range(0, TB, nst):
                    state = {}
                    nmm = nst * KB
                    for m in range(nmm):
                        def fn(m=m, st0=st0, state=state):
                            if m == 0:
                                state["t"] = psp_pool.tile(
                                    [128, nst * nch], f32, tag="psp", name="pspv")
                            i, kk = divmod(m, KB)
                            nc.tensor.matmul(
                                state["t"][:, i * nch:(i + 1) * nch],
                                xT_sb[:, kk, (st0 + i) * 128:(st0 + i + 1) * 128],
                                w_vt[:, kk, c0:c0 + nch],
                                start=(kk == 0), stop=(kk == KB - 1),
                            )
                            if m == nmm - 1:
                                emit_v_scatter(state["t"], st0, nst, c0, nch)
                        filler.add(nch, fn)
                    idx = filler.mark()
                    for st in range(st0, st0 + nst):
                        vmark[(cb, st)] = (filler, idx)

            def oproj_quanta(filler, cb, dst, addend, markd=None):
                # dst[mt] = outT[cb]^T @ wo[cb] + addend (bf16 staging)
                for mt in range(TB):
                    for half in range(2):
                        cols = slice(half * 384, half * 384 + 384)
                        def fn(mt=mt, cols=cols, cb=cb):
                            psp = psp_pool.tile([128, 384], f32, tag="psp",
                                                name="pspo")
                            nc.tensor.matmul(
                                psp, outT_sb[:, cb, mt * 128:(mt + 1) * 128],
                                w_ot[:, cb, cols], start=True, stop=True,
                            )
                            if addend is sOutA:
                                nc.vector.tensor_add(
                                    dst[:, mt, cols], psp, addend[:, mt, cols])
                            else:
                                nc.vector.tensor_add(dst[:, mt, cols], psp,
                                                     addend[:, cols])
                        filler.add(384, fn)
                        if markd is not None:
                            markd[(mt, half)] = (filler, filler.mark())

            def emit_pv(item):
                s, e, psC0, psC1, cb = item
                for hb, psC in ((0, psC0), (1, psC1)):
                    h = 2 * cb + hb
                    nc.tensor.matmul(
                        psC, v_sb[:, s, h * VW:(h + 1) * VW],
                        e[:, hb * 512:hb * 512 + 512],
                        start=(s == 0), stop=(s == TB - 1),
                    )

            def emit_drain(item, post_hb=None, direct=False):
                # per head: psC -> SBUF stage (frees the bank), DVE recip of
                # row 64, Pool partition_broadcast, DVE mul into outT.
                psC0, psC1, cb, jc = item
                jcols = slice(jc * 512, jc * 512 + 512)
                for hb, psC in ((0, psC0), (1, psC1)):
                    if direct:
                        # fp32 straight from PSUM (tail path)
                        nr0 = n_pool.tile([1, 512], f32, tag="nr0")
                        nrb = n_pool.tile([HD, 512], f32, tag="nrb")
                        nc.vector.reciprocal(nr0, psC[HD:VW, :])
                        nc.gpsimd.partition_broadcast(nrb, nr0)
                        nc.vector.tensor_mul(
                            outT_sb[64 * hb:64 * hb + 64, cb, jcols],
                            psC[0:HD, :], nrb)
                    else:
                        # bf16 stage: frees the PSUM bank AND makes the
                        # normalize mul eligible for the DVE 4x bf16 mode
                        stage = st_pool.tile([VW, 512], bf16, tag="st")
                        nc.vector.tensor_copy(stage, psC)
                        nr0 = n_pool.tile([1, 512], bf16, tag="nr0")
                        nrb = n_pool.tile([HD, 512], bf16, tag="nrb")
                        with nc.allow_low_precision(
                                reason="softmax denom at bf16; output is "
                                       "bf16 anyway, ~0.4% on the row scale"):
                            nc.vector.reciprocal(nr0, stage[HD:VW, :])
                            nc.gpsimd.partition_broadcast(nrb, nr0)
                            nc.vector.tensor_mul(
                                outT_sb[64 * hb:64 * hb + 64, cb, jcols],
                                stage[0:HD, :], nrb)
                    if post_hb is not None:
                        post_hb(hb)

            epi_osb = {}

            def emit_epi(mt, half):
                # cb2 projection term + staged partials -> bf16 partial out
                m = omark.get((mt, half))
                if m is not None:
                    m[0].pace_until(m[1])
                cols = slice(half * 384, half * 384 + 384)
                psp = psp_pool.tile([128, 384], f32, tag="psp", name="pspe")
                nc.tensor.matmul(
                    psp, outT_sb[:, 2, mt * 128:(mt + 1) * 128],
                    w_ot[:, 2, cols], start=True, stop=True,
                )
                if half == 0:
                    epi_osb[mt] = o_pool.tile([128, DIM], bf16, tag="osb",
                                              name="osb")
                osb = epi_osb[mt]
                nc.vector.tensor_add(osb[:, cols], psp, sOutB[:, mt, cols])
                if half == 1:
                    nc.sync.dma_start(
                        out=out_p.ap()[mt * 128:(mt + 1) * 128, :], in_=osb)
                    del epi_osb[mt]

            def emit_epi_full(mt, use_act, split=False):
                # cb2 term + identity fold of staged partials; the copy out of
                # PSUM alternates ACT/DVE so the two engines split the tail.
                # split=True (last tile only): per-half copy+DMA so the final
                # wire-out overlaps the remaining matmuls.
                psD = psB_pool.tile([128, DIM], f32, tag="psB", name="psD")
                osb = o_pool.tile([128, DIM], bf16, tag="osb", name="osbf")
                eng = nc.scalar.copy if use_act else nc.vector.tensor_copy
                for half, cols in enumerate((slice(0, 512), slice(512, DIM))):
                    nc.tensor.matmul(
                        psD[:, cols], outT_sb[:, 2, mt * 128:(mt + 1) * 128],
                        w_ot[:, 2, cols], start=True, stop=False,
                    )
                    nc.tensor.matmul(
                        psD[:, cols], ident, sOutB[:, mt, cols],
                        start=False, stop=True,
                    )
                    if split:
                        eng(osb[:, cols], psD[:, cols])
                        nc.sync.dma_start(
                            out=out_p.ap()[mt * 128:(mt + 1) * 128, cols],
                            in_=osb[:, cols])
                if not split:
                    eng(osb, psD)
                    nc.sync.dma_start(
                        out=out_p.ap()[mt * 128:(mt + 1) * 128, :], in_=osb)

            # ---- filler streams ----
            f0 = _Filler()
            # cb0 k/q weights come from the combined w_kq0 tile (loaded first)
            kq_quanta(f0, w_kq0[:, 0], kT_sb, bk_sb, 0, (1,), kmark)
            v_quanta(f0, 0)                     # v(cb0) g0-3, vmarks
            kq_quanta(f0, w_kq0[:, 0], kT_sb, bk_sb, 0, (2, 3), kmark)
            kq_quanta(f0, w_kq0[:, 1], qT_sb, bq_sb, 0, (1, 2, 3), qmark)
            kq_quanta(f0, w_kt, kT_sb, bk_sb, 1, (0, 1, 2, 3), kmark)
            kq_quanta(f0, w_qt, qT_sb, bq_sb, 1, (0,), qmark)
            v_quanta(f0, 1)
            f1 = _Filler()
            kq_quanta(f1, w_qt, qT_sb, bq_sb, 1, (1, 2, 3), qmark)
            kq_quanta(f1, w_kt, kT_sb, bk_sb, 2, (0, 1, 2, 3), kmark)
            kq_quanta(f1, w_qt, qT_sb, bq_sb, 2, (0,), qmark)
            v_quanta(f1, 2)
            kq_quanta(f1, w_qt, qT_sb, bq_sb, 2, (1, 2, 3), qmark)
            f2a = _Filler()
            oproj_quanta(f2a, 0, sOutA, bo_sb)
            f2b = _Filler()
            oproj_quanta(f2b, 1, sOutB, sOutA, omark)

            # ---- attention: 12 units of (head-pair cb, 512-col jc) ----
            pend = []       # (s, e, psC0, psC1, cb) awaiting PV
            dpend = []      # (psC0, psC1, cb, jc) awaiting drain
            epi_done = [0]  # halves emitted for mt 0-11 (24 total)

            def pop_pv():
                it = pend.pop(0)
                m = vmark.get((it[4], it[0]))
                if m is not None:
                    m[0].pace_until(m[1])
                emit_pv(it)
                if it[0] == TB - 1:
                    emit_drain(dpend.pop(0))

            gs = 0
            for cb in range(CB):
                for jc in range(NJC):
                    m = qmark.get((cb, jc))
                    if m is not None:
                        m[0].pace_until(m[1])
                    psC0 = psC_pool.tile([VW, 512], f32, tag="psC", name="psC0")
                    psC1 = psC_pool.tile([VW, 512], f32, tag="psC", name="psC1")
                    for s in range(TB):
                        if jc == 0 and s % 4 == 3 and s < 12:
                            # force k jt s//4+1: jt1 at s3, jt2 s7, jt3 s11
                            m = kmark.get((cb, s // 4 + 1))
                            if m is not None:
                                m[0].pace_until(m[1])
                        if s == 13:
                            # pre-force the NEXT unit's q chunk mid-unit so the
                            # burst doesn't collide with the boundary drains
                            nxt = (cb, jc + 1) if jc + 1 < NJC else (cb + 1, 0)
                            m = qmark.get(nxt)
                            if m is not None:
                                m[0].pace_until(m[1])
                        psB = psB_pool.tile([128, 1024], f32, tag="psB")
                        for hb in range(2):
                            prow = slice(64 * hb, 64 * hb + 64)
                            nc.tensor.matmul(
                                psB[:, hb * 512:hb * 512 + 512],
                                kT_sb[prow, cb, s * 128:(s + 1) * 128],
                                qT_sb[prow, cb, jc * 512:(jc + 1) * 512],
                                start=True, stop=True,
                            )
                        e = e_pool.tile([128, 1024], bf16, tag="e")
                        nc.scalar.activation(
                            e, psB, mybir.ActivationFunctionType.Exp,
                            scale=SCALE,
                        )
                        pend.append((s, e, psC0, psC1, cb))
                        thr = 1 if (cb == 2 and jc == 3) else 2
                        if len(pend) > thr:
                            pop_pv()
                            if len(pend) > thr:
                                pop_pv()
                        # background pacing (two half-steps for smoothness)
                        for ph in (0.5, 1.0):
                            if gs < 16:
                                f0.pace((gs + ph) / 36.0)
                            if gs < 80:
                                f0.pace((gs + ph) / 72.0)
                            if 80 <= gs < 136:
                                f1.pace((gs - 80 + ph) / 55.0)
                            if 68 <= gs < 133:
                                f2a.pace((gs - 68 + ph) / 64.0)
                            if gs >= 134:
                                f2b.pace((gs - 134 + ph) / 32.0)
                        # epilogue batches ride cb2 units jc>=1; the last 4
                        # halves (mt10-11) are held back to fill the PE hole
                        # under the final drain chain
                        if cb == 2 and jc >= 1 and s >= 4:
                            want = min(8, (s - 3) * 8 // 12)
                            cap = 22 if jc == 3 else (jc - 1) * 8 + 8
                            while epi_done[0] < min(cap, (jc - 1) * 8 + want):
                                emit_epi(epi_done[0] // 2, epi_done[0] % 2)
                                epi_done[0] += 1
                        gs += 1
                    dpend.append((psC0, psC1, cb, jc))
            f0.flush()
            f1.flush()
            f2a.flush()

            while pend:
                it = pend.pop(0)
                emit_pv(it)
                if it[0] == TB - 1:
                    # finale: the last drain runs as 256-col chunks (one per
                    # mt PAIR: recip/bcast/mul straight from PSUM, fewer
                    # cross-engine sem hops than per-mt) and each pair's
                    # epi_fulls start as soon as its columns are normalized
                    f2b.flush()
                    # mt11 held back from the stream: its identity-fold mms
                    # (no DVE coupling) fill the PE under the first drain chunk
                    emit_epi_full(11, use_act=True)
                    psC0f, psC1f, _cb, _jc = dpend.pop(0)
                    for mtp in (12, 14):
                        cc = slice((mtp - 12) * 128, (mtp - 12) * 128 + 256)
                        for hb, psC in ((0, psC0f), (1, psC1f)):
                            nr0 = n_pool.tile([1, 256], f32, tag="nr0")
                            nrb = n_pool.tile([HD, 256], f32, tag="nrb")
                            nc.vector.reciprocal(nr0, psC[HD:VW, cc])
                            nc.gpsimd.partition_broadcast(nrb, nr0)
                            nc.vector.tensor_mul(
                                outT_sb[64 * hb:64 * hb + 64, 2,
                                        mtp * 128:(mtp + 2) * 128],
                                psC[0:HD, cc], nrb)
                        emit_epi_full(mtp, use_act=(mtp % 2 == 1))
                        emit_epi_full(mtp + 1, use_act=(mtp % 2 == 0),
                                      split=(mtp == 14))


    nc.compile()
    return nc


def _get_program():
    if "nc" not in _PROGRAM_CACHE:
        _PROGRAM_CACHE["nc"] = _build_program()
    return _PROGRAM_CACHE["nc"]


def _prep_in_maps(inputs):
    f = np.float32
    inputs = {k: np.asarray(v) for k, v in inputs.items()}

    def eff(w, a, bl):
        return (w.astype(np.float64) + bl.astype(np.float64) @ a.astype(np.float64)).astype(f)

    wq = eff(inputs["wq"], inputs["laq"], inputs["lbq"])
    wk = eff(inputs["wk"], inputs["lak"], inputs["lbk"])
    wv = eff(inputs["wv"], inputs["lav"], inputs["lbv"])
    wo = eff(inputs["wo"], inputs["lao"], inputs["lbo"])
    x = np.asarray(inputs["x"], dtype=f)
    bq, bk, bv, bo = (np.asarray(inputs[k], dtype=f) for k in ("bq", "bk", "bv", "bo"))

    in_maps = []
    for core in range(NCORES):
        b, g = core // 2, core % 2
        cols = slice(g * CS, (g + 1) * CS)
        bo_core = wo[:, cols].astype(np.float64) @ bv[cols].astype(np.float64)
        if g == 0:
            bo_core = bo_core + bo
        in_maps.append({
            "xT": _bf16(x[b].T),
            "wq_t": _bf16(_pack_w(wq[cols, :].T)),
            "wkq0": _bf16(np.concatenate(
                [_pack_w(wk[cols, :].T)[:, 0:DIM],
                 _pack_w(wq[cols, :].T)[:, 0:DIM]], axis=1)),
            "wk_t": _bf16(_pack_w(wk[cols, :].T)),
            "wv_t": _bf16(_pack_w(wv[cols, :].T)),
            "wo_t": _bf16(wo[:, cols].T),
            "bq_s": np.ascontiguousarray(bq[cols]),
            "bk_s": np.ascontiguousarray(bk[cols]),
            "bo_s": bo_core.astype(f),
            "ident": _bf16(np.eye(128, dtype=f)),
        })
    return in_maps


def kernel(**inputs):
    from concourse.bass_utils import run_bass_kernel_spmd

    nc = _get_program()
    in_maps = _prep_in_maps(inputs)
    res = run_bass_kernel_spmd(nc, in_maps, core_ids=list(range(NCORES)))
    out = np.empty((B, T, DIM), dtype=np.float32)
    for b in range(B):
        out[b] = (res.results[2 * b]["out_p"].astype(np.float32)
                  + res.results[2 * b + 1]["out_p"].astype(np.float32))
    return out
